# revision 28
# baseline (speedup 1.0000x reference)
"""Trainium2 Bass kernel for BasePropagationGraphPositionalEncoding.

Computes, for each batch element b:
    out[b] = (sum_k coefs[k] * gr_kernel[b, k]) @ x[b] / sum_k coefs[k]
with coefs[k] = (1 - EPS)^k, EPS = 0.01, K = 9.

Sharding: batch dim B=8 across the 8 NeuronCores (data parallel, no
cross-core communication). Each core streams its 36 MB of gr_kernel slabs
from HBM (the memory-bound term), does the weighted k-sum on VectorE
(fused multiply-accumulate via scalar_tensor_tensor), transposes the
summed [128,128] tiles on TensorE (fp32 transpose mode), and contracts
against x with PSUM-accumulated matmuls.
"""

import sys

if "/opt/trn_rl_repo" not in sys.path:
    sys.path.insert(0, "/opt/trn_rl_repo")

import numpy as np

import concourse.bass as bass
import concourse.mybir as mybir
from concourse import tile
from concourse.bacc import Bacc
from concourse.masks import make_identity
from concourse.bass_utils import run_bass_kernel_spmd

# Problem shapes (hardcoded per the harness contract).
B, K, N, D = 8, 9, 1024, 64
EPS = 0.01
P = 128          # SBUF partitions
NT = N // P      # 8 row/col tiles of the [N, N] kernel

F32 = mybir.dt.float32


def build_bass() -> bass.Bass:
    # Bacc (not plain Bass): its compile() runs generate_event_semaphores /
    # move_matmul_waits_to_ldweights, splitting multi-semaphore waits that
    # the 64B ISA instructions (single EVENTS slot) cannot carry.
    nc = Bacc()

    x_d = nc.dram_tensor("x_b", (N, D), F32, kind="ExternalInput")
    g_d = nc.dram_tensor("g_b", (K, N, N), F32, kind="ExternalInput")
    o_d = nc.dram_tensor("out_b", (N, D), F32, kind="ExternalOutput")

    coefs = (1.0 - EPS) ** np.arange(K, dtype=np.float64)
    w = coefs / coefs.sum()  # fold the 1/sum normalization into the k-sum

    with tile.TileContext(nc) as tc:
        with (
            tc.tile_pool(name="consts", bufs=1) as consts,
            tc.tile_pool(name="gr", bufs=2) as gr_pool,
            tc.tile_pool(name="wk", bufs=2) as wk_pool,
            tc.tile_pool(name="wkt", bufs=4) as wkt_pool,
            tc.tile_pool(name="outp", bufs=2) as out_pool,
            tc.tile_pool(name="ps_t", bufs=4, space=bass.MemorySpace.PSUM) as ps_t,
            tc.tile_pool(name="ps_e", bufs=2, space=bass.MemorySpace.PSUM) as ps_e,
        ):
            # Identity for TensorE transpose. Built by GPSIMD, then copied
            # through VectorE so the first PE transpose waits on a single
            # semaphore (DVE) — Matmult lowering only supports one sync wait.
            ident_raw = consts.tile([P, P], F32)
            make_identity(nc, ident_raw)
            ident = consts.tile([P, P], F32)
            nc.vector.tensor_copy(ident[:], ident_raw[:])

            # Per-band, per-slab loads: slab k of band i is its own tile and
            # its own contiguous 512 KB DMA. Separate tiles are essential:
            # slab-DMAs into slices of a shared tile get WAW-serialized by
            # Tile (one DMA in flight -> stream drops from 388 to 333 GB/s),
            # and per-slab deps let the VectorE k-sum start as soon as slab 0
            # lands.
            def load_band(i):
                tiles = []
                for k in range(K):
                    g_k = gr_pool.tile([P, N], F32, tag=f"g{k}")
                    nc.sync.dma_start(g_k[:], g_d[k, i * P : (i + 1) * P, :])
                    tiles.append(g_k)
                return tiles

            band_tiles = load_band(0)

            # x rearranged to [p, chunk, d] so chunk c is a [128, 64] tile
            # with the contraction index m = c*128 + p on partitions.
            # Loaded after band 0's stream is issued — x isn't needed until
            # the first emb matmul.
            x_sb = consts.tile([P, NT, D], F32)
            nc.gpsimd.dma_start(x_sb[:], x_d.rearrange("(c p) d -> p c d", p=P))

            for i in range(NT):
                g_ts = band_tiles
                if i + 1 < NT:
                    band_tiles = load_band(i + 1)

                # Weighted k-sum on VectorE: wk = sum_k w[k] * slab_k.
                wk = wk_pool.tile([P, N], F32)
                nc.vector.tensor_scalar_mul(wk[:], g_ts[0][:], float(w[0]))
                for k in range(1, K):
                    nc.vector.scalar_tensor_tensor(
                        wk[:],
                        g_ts[k][:],
                        float(w[k]),
                        wk[:],
                        op0=mybir.AluOpType.mult,
                        op1=mybir.AluOpType.add,
                    )

                # Transpose the 8 [128,128] tiles of wk on TensorE in quads
                # (one PSUM bank each), staging to SBUF with one ACT copy
                # per quad.
                wkT_sb = wkt_pool.tile([P, NT, P], F32)
                for q in range(NT // 4):
                    wkT_ps = ps_t.tile([P, 4, P], F32)
                    for j in range(4):
                        c = q * 4 + j
                        nc.tensor.transpose(
                            wkT_ps[:, j, :], wk[:, c * P : (c + 1) * P], ident[:]
                        )
                    nc.scalar.copy(wkT_sb[:, q * 4 : q * 4 + 4, :], wkT_ps[:])

                # emb[i-band] = sum_c wk_tile(i,c) @ x_chunk(c), accumulated
                # in PSUM over the 8 contraction chunks.
                emb_ps = ps_e.tile([P, D], F32)
                for c in range(NT):
                    nc.tensor.matmul(
                        emb_ps[:],
                        wkT_sb[:, c, :],
                        x_sb[:, c, :],
                        start=(c == 0),
                        stop=(c == NT - 1),
                    )

                o_sb = out_pool.tile([P, D], F32)
                nc.scalar.copy(o_sb[:], emb_ps[:])
                nc.gpsimd.dma_start(o_d[i * P : (i + 1) * P, :], o_sb[:])

    nc.compile()
    return nc


_NC = None


def _get_nc() -> bass.Bass:
    global _NC
    if _NC is None:
        _NC = build_bass()
    return _NC


def run(x: np.ndarray, gr_kernel: np.ndarray, **spmd_kwargs):
    """Run the SPMD kernel on cores 0-7; returns BassKernelResults."""
    nc = _get_nc()
    in_maps = [
        {
            "x_b": np.ascontiguousarray(x[b], dtype=np.float32),
            "g_b": np.ascontiguousarray(gr_kernel[b], dtype=np.float32),
        }
        for b in range(B)
    ]
    return run_bass_kernel_spmd(nc, in_maps, core_ids=list(range(B)), **spmd_kwargs)


def kernel(x: np.ndarray, gr_kernel: np.ndarray) -> np.ndarray:
    res = run(np.asarray(x), np.asarray(gr_kernel))
    out = np.stack([res.results[b]["out_b"] for b in range(B)], axis=0)
    return out.astype(np.float32, copy=False)


if __name__ == "__main__":
    rng = np.random.default_rng(0)
    x = rng.standard_normal((B, N, D), dtype=np.float32)
    g = rng.standard_normal((B, K, N, N), dtype=np.float32)
    out = kernel(x, g)
    coefs = (1.0 - EPS) ** np.arange(K)
    wk = np.einsum("k,bknm->bnm", coefs, g)
    ref = np.matmul(wk, x) / coefs.sum()
    err = np.linalg.norm(out - ref) / np.linalg.norm(ref)
    print("self-check rel err:", err)
